# revision 31
# baseline (speedup 1.0000x reference)
"""Multi-head attention (dense_transformer) Trainium2 Bass kernel.

Problem: x[8, 512, 32, 32]; per-batch 1x1-conv QKV projections, 8-head
attention over N=H*W=1024 positions (head_dim 64), output projection,
residual. Sharding: data-parallel over batch B=8 across the 8 cores --
one batch element per core, no collectives.

Key design points (v2, fp8-DoubleRow rewrite):
  - All projection / AV / output matmuls run in fp8e4m3 DoubleRow mode:
    one instruction contracts two 128-deep k-tiles at 0.5 cycles/row,
    4x the throughput of the bf16 accumulation chains it replaces.
  - Bias algebra: bk is dropped outright (softmax is shift-invariant
    along j, so the K bias cancels exactly); bv is folded into bo on the
    host (bo' = bo + Wo@bv, exact since attention rows sum to 1); bo'
    rides in the precomputed residual tensor xr = x + bo'; bq is added
    by a tiny [1-partition] DoubleRow ones-row matmul into the Q psum.
    No per-element bias vector ops remain on the device.
  - Q/K stay bf16 for the S = K^T Q matmuls (contraction is only 64
    deep -- fp8 DoubleRow does not apply -- and bf16 runs at the same
    1 cycle/row while keeping S accurate).
  - The softmax exp of the [1024, 1024] S matrix per head (8.4M
    elements -- the single largest engine workload) is split between
    the Activation engine (exact exp -> fp8 out) and the Vector engine
    (a one-instruction Schraudolph bit-trick: int8(S*A + B) aliased as
    fp8e4m3 approximates exp(S/8) to ~3% RMS, well inside the 2e-2
    tolerance).
  - Softmax denominators come free as a 65th ones-column of V^T in the
    AV matmul; gpsimd broadcasts the psum denominator row across 64
    partitions and the normalization is a single tensor_tensor divide
    (no reciprocal ops, no DRAM bounce).
  - GPSIMD (Pool) engine carries V-copies, denominator broadcasts and
    part of the normalize/residual work under the 'proxy' library.
"""

import sys

if "/opt/trn_rl_repo" not in sys.path:
    sys.path.insert(0, "/opt/trn_rl_repo")

import numpy as np
import ml_dtypes

import concourse.bass as bass
import concourse.mybir as mybir
from concourse.tile import TileContext

DIM = 512
NH = 8
HD = 64
N = 1024
P = 128
CT = DIM // P   # 4 c-tiles of 128 channels
JT = N // P     # 8 j-tiles of 128 positions
F32 = mybir.dt.float32
BF16 = mybir.dt.bfloat16
F8 = mybir.dt.float8e4
I8 = mybir.dt.int8
AOP = mybir.AluOpType
EXP = mybir.ActivationFunctionType.Exp
DRM = mybir.MatmulPerfMode.DoubleRow

# Schraudolph exp: fp8e4m3 byte ~= int8(S * SCH_A + SCH_B)  approximates
# exp(S * 0.125). B tuned numerically for truncating float->int casts.
SCH_A = 8.0 / np.log(2.0) * 0.125
SCH_B = 56.08

# GPSIMD (Pool) cannot touch PSUM (BIR verifier rule), so every psum
# drain runs on ACT or DVE; Pool only gets SBUF->SBUF work (denominator
# broadcast + normalize divide on the ACT-copied AV output).

# Exp engine split: within each jt the two head-tiles go to DIFFERENT
# engines (h2=0 -> ACT, h2=1 -> DVE) so both engines chew in parallel on
# the psS bufs; extra h2=0 tiles shift to DVE for load balance.
EXP_DVE = {(p, 1, jt) for p in range(4) for jt in range(8)}

# engine for each Q/K psum->bf16 copy, by (tensor, ot)
QK_COPY_ENG = {("q", 0): "A", ("k", 0): "V", ("q", 1): "A", ("k", 1): "V",
               ("q", 2): "A", ("k", 2): "V", ("q", 3): "A", ("k", 3): "V"}
# engine for the normalize divide per (head, ih): mid heads on Pool
# (reading the ACT-copied SBUF oraw), tail heads on the then-idle DVE.
NORM_ENG = {(h, ih): "P" for h in range(6) for ih in range(2)}
NORM_ENG.update({(6, 0): "V", (6, 1): "V", (7, 0): "V", (7, 1): "V"})
# residual per (ot, nh): "V" = DVE tensor_tensor straight from psum;
# "AP" = ACT copy to SBUF then Pool add (ACT and Pool are free late).
RESID_ENG = {(0, 0): "V", (0, 1): "AP", (1, 0): "V", (1, 1): "AP",
             (2, 0): "AP", (2, 1): "V", (3, 0): "AP", (3, 1): "V"}

# head -> (g, s, half) slot in the output-projection rhs. The host
# permutes Wo's input-channel order to match (see _prep_maps). Chosen so
# the tail heads h6/h7 land in lower halves (direct engine write, no
# partition-remap DMA on the critical tail); the remapped (half=1) heads
# h1/h3/h4/h5 all complete mid-kernel.
HEAD_SLOT = {0: (0, 0, 0), 1: (0, 0, 1), 2: (0, 1, 0), 3: (0, 1, 1),
             4: (1, 0, 1), 5: (1, 1, 1), 6: (1, 1, 0), 7: (1, 0, 0)}


class FixedTileContext(TileContext):
    """Works around a walrus/bass snapshot mismatch: this walrus build
    accepts only one sync-wait command per instruction, but Tile's wait
    assigner happily attaches several. After scheduling, excess waits on
    any instruction are peeled off onto same-engine NOPs inserted right
    before it (same blocking semantics: the engine executes in order)."""

    MAX_WAITS = 1
    MAX_WAITS_DATA = 1
    _wsplit_ctr = 0

    def _split_sync_waits(self):
        seq_only = mybir.SEQUENCER_ONLY_OPCODES
        for fn in self.nc.m.functions:
            for blk in fn.blocks:
                insts = list(blk.instructions)
                out = []
                for inst in insts:
                    si = inst.sync_info
                    limit = (
                        self.MAX_WAITS
                        if inst.opcode in seq_only
                        else self.MAX_WAITS_DATA
                    )
                    if si is not None and len(si.on_wait) > limit:
                        waits = list(si.on_wait)
                        movers = waits[:-limit]
                        keep = waits[-limit:]
                        del si.on_wait[:]
                        for w in keep:
                            si.on_wait.append(w)
                        for w in movers:
                            FixedTileContext._wsplit_ctr += 1
                            nop = mybir.InstNoOp(
                                name=f"wsplit-{FixedTileContext._wsplit_ctr}",
                                ins=[],
                                outs=[],
                            )
                            nop.engine = inst.engine
                            nop.sync_info = mybir.SyncInfo(on_wait=[w], on_update=[])
                            out.append(nop)
                    out.append(inst)
                if len(out) != len(insts):
                    del blk.instructions[:]
                    for i in out:
                        blk.add_instruction(i)

    split_on_exit = True

    def __exit__(self, *exc):
        ret = super().__exit__(*exc)
        if exc[0] is None and self.split_on_exit:
            self._split_sync_waits()
        return ret


def build_nc(split_waits=True):
    nc = bass.Bass()

    x8d = nc.dram_tensor("x8", [P, CT, N], F8, kind="ExternalInput")
    xrd = nc.dram_tensor("xr32", [P, CT, N], F32, kind="ExternalInput")
    wq8d = nc.dram_tensor("wq8", [P, CT, DIM], F8, kind="ExternalInput")
    wk8d = nc.dram_tensor("wk8", [P, CT, DIM], F8, kind="ExternalInput")
    wv8d = nc.dram_tensor("wv8", [P, CT, DIM], F8, kind="ExternalInput")
    wo8d = nc.dram_tensor("wo8", [P, CT, DIM], F8, kind="ExternalInput")
    bq8d = nc.dram_tensor("bq8", [1, 2, DIM], F8, kind="ExternalInput")
    on8d = nc.dram_tensor("on8", [1, 2, N], F8, kind="ExternalInput")
    outd = nc.dram_tensor("out", [DIM, N], BF16, kind="ExternalOutput")
    outr = outd.rearrange("(t p) n -> t p n", p=P)

    FixedTileContext.split_on_exit = split_waits
    with FixedTileContext(nc) as tc:
        with (
            tc.tile_pool(name="persist", bufs=1) as persist,
            tc.tile_pool(name="small", bufs=3) as small,
            tc.tile_pool(name="dram", bufs=1, space="DRAM") as dram,
            tc.tile_pool(name="obf", bufs=8) as obf,
            tc.tile_pool(name="psS", bufs=2, space="PSUM") as psS,
        ):
            # persistent SBUF tensors
            x8 = persist.tile([P, CT, N], F8, tag="x8", name="x8")
            xr32 = persist.tile([P, CT, N], F32, tag="xr32", name="xr32")
            wq8 = persist.tile([P, CT, DIM], F8, tag="wq8", name="wq8")
            wk8 = persist.tile([P, CT, DIM], F8, tag="wk8", name="wk8")
            wv8 = persist.tile([P, CT, DIM], F8, tag="wv8", name="wv8")
            wo8 = persist.tile([P, CT, DIM], F8, tag="wo8", name="wo8")
            bq8 = persist.tile([1, 2, DIM], F8, tag="bq8", name="bq8")
            on8 = persist.tile([1, 2, N], F8, tag="on8", name="on8")
            qt = [persist.tile([P, N], BF16, tag=f"q_{t}", name=f"q_{t}")
                  for t in range(CT)]
            kt = [persist.tile([P, N], BF16, tag=f"k_{t}", name=f"k_{t}")
                  for t in range(CT)]
            vt = [persist.tile([P, 2, NH, HD + 2], F8, tag=f"v_{a}",
                               name=f"v_{a}") for a in range(CT)]
            p8 = {}
            for a in range(CT):
                for h in range(NH):
                    p8[(a, h)] = persist.tile(
                        [P, 2, N], F8, tag=f"p8_{a}_{h}", name=f"p8_{a}_{h}")
            o8 = [persist.tile([P, 2, N], F8, tag=f"o8_{g}", name=f"o8_{g}")
                  for g in range(2)]
            ones8 = persist.tile([P, 2, HD], F8, tag="ones8", name="ones8")
            nc.vector.memset(ones8, 1.0)

            def copy_psum(eng, out_ap, in_ap):
                if eng == "A":
                    nc.scalar.copy(out_ap, in_ap)
                else:
                    nc.vector.tensor_copy(out_ap, in_ap)

            def exp_op(key, dst_ap, src_ap):
                if key in EXP_DVE:
                    nc.vector.tensor_scalar(dst_ap.bitcast(I8), src_ap,
                                            SCH_A, SCH_B, AOP.mult, AOP.add)
                else:
                    nc.scalar.activation(dst_ap, src_ap, EXP, scale=0.125)

            # S tiles cycle through 3 slots (2 in psS + 1 in psS2 once the
            # projection pool has closed) so both exp engines always have a
            # tile in flight and a third is being refilled by the PE.
            s_slot = [0]

            def s_tile(p, h2, jt, pools):
                h = 2 * p + h2
                pool, ptag = pools[s_slot[0] % len(pools)]
                s_slot[0] += 1
                t = pool.tile([P, N], F32, tag=ptag, name=f"s_{p}_{h2}_{jt}")
                for ih in range(2):
                    nc.tensor.matmul(
                        t[:, ih * 512:(ih + 1) * 512],
                        lhsT=kt[p][64 * h2:64 * h2 + 64, jt * P:(jt + 1) * P],
                        rhs=qt[p][64 * h2:64 * h2 + 64, ih * 512:(ih + 1) * 512],
                        start=True, stop=True)
                exp_op((p, h2, jt), p8[(jt // 2, h)][:, jt % 2, :], t)

            with tc.tile_pool(name="pp", bufs=2, space="PSUM") as pp:
                # ---- input DMAs ---- (ACT queue kept clear for copies/exps)
                nc.sync.dma_start(out=wq8, in_=wq8d[:, :, :])
                nc.sync.dma_start(out=x8[:, 0:2, :], in_=x8d[:, 0:2, :])
                nc.gpsimd.dma_start(out=x8[:, 2:4, :], in_=x8d[:, 2:4, :])
                nc.sync.dma_start(out=wk8, in_=wk8d[:, :, :])
                nc.scalar.dma_start(out=bq8, in_=bq8d[:, :, :])
                nc.scalar.dma_start(out=on8, in_=on8d[:, :, :])
                nc.sync.dma_start(out=wv8, in_=wv8d[:, :, :])

                # warm the exp table on ACT before the first real exp
                warm = small.tile([1, 8], F32, tag="warm", name="warm")
                nc.vector.memset(warm, 0.0)
                nc.scalar.activation(warm, warm, EXP)

                def proj_qk(w8, ot, with_bias, eng, name):
                    ps = pp.tile([P, N], F32, tag="pp", name=f"pp_{name}{ot}")
                    for nh in range(2):
                        sl = slice(nh * 512, (nh + 1) * 512)
                        for g in range(2):
                            nc.tensor.matmul(
                                ps[:, sl],
                                lhsT=w8[:, 2 * g:2 * g + 2, ot * P:(ot + 1) * P],
                                rhs=x8[:, 2 * g:2 * g + 2, sl],
                                start=(g == 0),
                                stop=(g == 1),
                                perf_mode=DRM)
                        if with_bias:
                            nc.tensor.matmul(
                                ps[:, sl],
                                lhsT=bq8[:, :, ot * P:(ot + 1) * P],
                                rhs=on8[:, :, sl],
                                start=False, stop=True, perf_mode=DRM,
                                skip_group_check=True)
                    dst = qt[ot] if name == "q" else kt[ot]
                    copy_psum(eng, dst[:, :], ps[:, :])

                def proj_v(a):
                    ps = pp.tile([P, N], F32, tag="pp", name=f"pp_v{a}")
                    for jloc in range(2):
                        jt = 2 * a + jloc
                        sl = slice(jloc * 512, (jloc + 1) * 512)
                        for g in range(2):
                            nc.tensor.matmul(
                                ps[:, sl],
                                lhsT=x8[:, 2 * g:2 * g + 2, jt * P:(jt + 1) * P],
                                rhs=wv8[:, 2 * g:2 * g + 2, :],
                                start=(g == 0), stop=(g == 1), perf_mode=DRM)
                    psv = ps.rearrange("p (s h d) -> p s h d", s=2, h=NH)
                    if a % 2 == 0:
                        nc.scalar.copy(vt[a][:, :, :, 0:HD], psv)
                    else:
                        nc.vector.tensor_copy(vt[a][:, :, :, 0:HD], psv)
                    nc.vector.memset(vt[a][:, :, :, HD:HD + 2], 1.0)

                proj_qk(wq8, 0, True, QK_COPY_ENG[("q", 0)], "q")
                proj_qk(wk8, 0, False, QK_COPY_ENG[("k", 0)], "k")
                for jt in range(2):
                    s_tile(0, 0, jt, [(psS, "psS")])
                    s_tile(0, 1, jt, [(psS, "psS")])
                proj_qk(wq8, 1, True, QK_COPY_ENG[("q", 1)], "q")
                proj_qk(wk8, 1, False, QK_COPY_ENG[("k", 1)], "k")
                for jt in range(2, JT):
                    s_tile(0, 0, jt, [(psS, "psS")])
                    s_tile(0, 1, jt, [(psS, "psS")])
                proj_qk(wq8, 2, True, QK_COPY_ENG[("q", 2)], "q")
                proj_qk(wk8, 2, False, QK_COPY_ENG[("k", 2)], "k")
                for a in range(CT):
                    proj_v(a)
                proj_qk(wq8, 3, True, QK_COPY_ENG[("q", 3)], "q")
                proj_qk(wk8, 3, False, QK_COPY_ENG[("k", 3)], "k")

                # late loads for the output stage
                nc.gpsimd.dma_start(out=wo8, in_=wo8d[:, :, :])
                nc.sync.dma_start(out=xr32, in_=xrd[:, :, :])

            def av_head(h, psO, rdram):
                # Mid-kernel heads: ACT drains the AV psum (head output +
                # denominator row) to SBUF; the denominator row bounces
                # through DRAM to become a 64-partition broadcast (DMA APs
                # allow the 0-stride partition dim engines reject) and the
                # normalize divide runs SBUF-only on Pool.
                g, s, half = HEAD_SLOT[h]
                odd = half == 1
                osc = None
                if odd:
                    osc = small.tile([HD, N], F8, tag="osc", name=f"osc_{h}")
                for ih in range(2):
                    sl = slice(ih * 512, (ih + 1) * 512)
                    po = psO.tile([HD + 2, 512], F32, tag="psO",
                                  name=f"po_{h}_{ih}")
                    for a in range(CT):
                        nc.tensor.matmul(
                            po, lhsT=vt[a][:, :, h, :],
                            rhs=p8[(a, h)][:, :, sl],
                            start=(a == 0), stop=(a == CT - 1),
                            perf_mode=DRM, skip_group_check=True)
                    oraw = small.tile([HD + 1, 512], F32, tag="oraw",
                                      name=f"or_{h}_{ih}")
                    nc.scalar.copy(oraw[:, :], po[0:HD + 1, :])
                    r = 2 * h + ih
                    nc.sync.dma_start(out=rdram[r:r + 1, :],
                                      in_=oraw[HD:HD + 1, :])
                    rb = small.tile([HD, 512], F32, tag="rb",
                                    name=f"rb_{h}_{ih}")
                    rsrc = rdram[r:r + 1, :]
                    nc.sync.dma_start(
                        out=rb[:, :],
                        in_=bass.AP(tensor=rsrc.tensor, offset=rsrc.offset,
                                    ap=[[0, HD]] + list(rsrc.ap[1:])))
                    dst_ap = osc[:, sl] if odd else o8[g][0:HD, s, sl]
                    nc.gpsimd.tensor_tensor(dst_ap, oraw[0:HD, :], rb[:, :],
                                            AOP.divide)
                if odd:
                    nc.sync.dma_start(out=o8[g][HD:P, s, :], in_=osc[:, :])

            def av_tail(h, psO, den_pool, den_tag):
                # Tail heads (h6/h7): the denominator broadcast comes from
                # the PE itself -- an all-ones lhsT DoubleRow accumulation
                # replicates the column sums across 64 psum partitions --
                # so the critical chain is just ACT-copy + DVE divide, with
                # no DRAM bounce latency.
                g, s, _ = HEAD_SLOT[h]
                td = den_pool.tile([P, N], F32, tag=den_tag, name=f"td_{h}")
                for ih in range(2):
                    sl = slice(ih * 512, (ih + 1) * 512)
                    po = psO.tile([HD + 2, 512], F32, tag="psO",
                                  name=f"po_{h}_{ih}")
                    for a in range(CT):
                        nc.tensor.matmul(
                            po, lhsT=vt[a][:, :, h, :],
                            rhs=p8[(a, h)][:, :, sl],
                            start=(a == 0), stop=(a == CT - 1),
                            perf_mode=DRM, skip_group_check=True)
                        nc.tensor.matmul(
                            td[0:HD, sl], lhsT=ones8[:, :, :],
                            rhs=p8[(a, h)][:, :, sl],
                            start=(a == 0), stop=(a == CT - 1),
                            perf_mode=DRM, skip_group_check=True)
                    den = small.tile([HD, 512], F32, tag="rb",
                                     name=f"den_{h}_{ih}")
                    nc.scalar.copy(den[:, :], td[0:HD, sl])
                    nc.vector.tensor_tensor(o8[g][0:HD, s, sl], po[0:HD, :],
                                            den[:, :], AOP.divide)

            def op_mm(ot, g, ps):
                # one g-layer of the output projection for both n-halves;
                # g-major emission lets the g0 layers run (and warm the PE
                # p-state) while the last heads' normalize is still going.
                for nh in range(2):
                    sl = slice(nh * 512, (nh + 1) * 512)
                    nc.tensor.matmul(
                        ps[:, sl],
                        lhsT=wo8[:, 2 * g:2 * g + 2, ot * P:(ot + 1) * P],
                        rhs=o8[g][:, :, sl],
                        start=(g == 0), stop=(g == 1),
                        perf_mode=DRM, skip_group_check=True)

            def op_tail(ot, ps):
                # residual + writeback per half: finer tail staggering.
                # "V": DVE adds straight from psum. "AP": ACT drains psum
                # to SBUF, Pool does the SBUF-only add (both free late).
                for nh in range(2):
                    sl = slice(nh * 512, (nh + 1) * 512)
                    ob = obf.tile([P, 512], BF16, tag="ob", name=f"ob_{ot}_{nh}")
                    if RESID_ENG[(ot, nh)] == "V":
                        nc.vector.tensor_tensor(ob[:, :], ps[:, sl],
                                                xr32[:, ot, sl], AOP.add)
                    else:
                        oc = obf.tile([P, 512], F32, tag="oc",
                                      name=f"oc_{ot}_{nh}")
                        nc.scalar.copy(oc[:, :], ps[:, sl])
                        nc.gpsimd.tensor_tensor(ob[:, :], oc[:, :],
                                                xr32[:, ot, sl], AOP.add)
                    nc.sync.dma_start(out=outr[ot][:, sl], in_=ob)

            with (
                tc.tile_pool(name="psS2", bufs=1, space="PSUM") as psS2,
                tc.tile_pool(name="psO", bufs=2, space="PSUM") as psO,
            ):
                pools3 = [(psS, "psS"), (psS, "psS"), (psS2, "psS2")]
                for jt in range(JT):
                    s_tile(1, 0, jt, pools3)
                    s_tile(1, 1, jt, pools3)
                rdram = dram.tile([12, 512], F32, tag="rdram", name="rdram")
                av_head(0, psO, rdram)
                av_head(1, psO, rdram)
                for jt in range(JT):
                    s_tile(2, 0, jt, pools3)
                    s_tile(2, 1, jt, pools3)
                av_head(2, psO, rdram)
                av_head(3, psO, rdram)
                for jt in range(JT):
                    s_tile(3, 1, jt, pools3)
                    s_tile(3, 0, jt, pools3)
                av_head(4, psO, rdram)
                av_head(5, psO, rdram)
                av_tail(7, psO, psS2, "psS2")
                av_tail(6, psO, psS, "psS")
                ps0 = psS.tile([P, N], F32, tag="psS", name="op_0")
                ps1 = psS.tile([P, N], F32, tag="psS", name="op_1")
                op_mm(0, 0, ps0)
                op_mm(1, 0, ps1)
                op_mm(0, 1, ps0)
                op_mm(1, 1, ps1)
                op_tail(0, ps0)
                op_tail(1, ps1)
            with tc.tile_pool(name="po3", bufs=2, space="PSUM") as po3:
                ps2 = po3.tile([P, N], F32, tag="po3", name="op_2")
                ps3 = po3.tile([P, N], F32, tag="po3", name="op_3")
                op_mm(2, 0, ps2)
                op_mm(3, 0, ps3)
                op_mm(2, 1, ps2)
                op_mm(3, 1, ps3)
                op_tail(2, ps2)
                op_tail(3, ps3)
    return nc


_BF = ml_dtypes.bfloat16
_E4 = ml_dtypes.float8_e4m3


def _prep_maps(x, Wq, bq, Wk, bk, Wv, bv, Wo, bo):
    # plain numpy up front: inputs may arrive as jax device arrays and
    # transforming those would trigger on-device jax execution
    x, Wq, bq, Wk, bk, Wv, bv, Wo, bo = (
        np.asarray(a, dtype=np.float32)
        for a in (x, Wq, bq, Wk, bk, Wv, bv, Wo, bo)
    )
    B, C, H, W = x.shape
    xf = np.ascontiguousarray(x.reshape(B, C, H * W))

    def wtile(Wm):
        # [128, CT, DIM] with [p, t, o] = W[o, 128t + p], fp8
        return np.ascontiguousarray(
            Wm.T.reshape(CT, P, DIM).transpose(1, 0, 2)).astype(_E4)

    # Wo's input channels are permuted to match the device's o8 head-slot
    # layout (HEAD_SLOT): channel 128*(2g+s) + 64*half + d <- head H's d.
    cperm = np.zeros(DIM, np.int64)
    for h, (g, s, half) in HEAD_SLOT.items():
        base = P * (2 * g + s) + HD * half
        cperm[base:base + HD] = HD * h + np.arange(HD)
    Wo_p = Wo[:, cperm]

    bo_p = bo + Wo @ bv  # bv folded through the output projection (exact)
    bq8 = np.zeros((1, 2, DIM), np.float32)
    bq8[0, 0, :] = 64.0 * bq
    on8 = np.zeros((1, 2, N), np.float32)
    on8[0, 0, :] = 1.0 / 64.0
    shared = {
        "wq8": wtile(Wq), "wk8": wtile(Wk), "wv8": wtile(Wv),
        "wo8": wtile(Wo_p), "bq8": bq8.astype(_E4), "on8": on8.astype(_E4),
    }
    in_maps = []
    for b in range(B):
        m = dict(shared)
        xb = xf[b]                                  # [C, N]
        m["x8"] = np.ascontiguousarray(
            xb.reshape(CT, P, N).transpose(1, 0, 2)).astype(_E4)
        m["xr32"] = np.ascontiguousarray(
            (xb + bo_p[:, None]).reshape(CT, P, N).transpose(1, 0, 2))
        in_maps.append(m)
    return in_maps


def kernel(x, Wq, bq, Wk, bk, Wv, bv, Wo, bo, _trace=False):
    from concourse.bass_utils import run_bass_kernel_spmd

    x = np.asarray(x)
    B, C, H, W = x.shape
    in_maps = _prep_maps(x, Wq, bq, Wk, bk, Wv, bv, Wo, bo)
    nc = build_nc()
    res = run_bass_kernel_spmd(nc, in_maps, core_ids=list(range(B)), trace=_trace)
    out = np.stack([res.results[b]["out"] for b in range(B)])
    out = out.reshape(B, C, H, W).astype(np.float32)
    if _trace:
        kernel.last_results = res
    return out


# revision 72
# speedup vs baseline: 1.0404x; 1.0404x over previous
"""Multi-head attention (dense_transformer) Trainium2 Bass kernel.

Problem: x[8, 512, 32, 32]; per-batch 1x1-conv QKV projections, 8-head
attention over N=H*W=1024 positions (head_dim 64), output projection,
residual. Sharding: data-parallel over batch B=8 across the 8 cores --
one batch element per core, no collectives.

Key design points (v2, fp8-DoubleRow rewrite):
  - All projection / AV / output matmuls run in fp8e4m3 DoubleRow mode:
    one instruction contracts two 128-deep k-tiles at 0.5 cycles/row,
    4x the throughput of the bf16 accumulation chains it replaces.
  - Bias algebra: bk is dropped outright (softmax is shift-invariant
    along j, so the K bias cancels exactly); bv is folded into bo on the
    host (bo' = bo + Wo@bv, exact since attention rows sum to 1); bo'
    rides in the precomputed residual tensor xr = x + bo'; bq is added
    by a tiny [1-partition] DoubleRow ones-row matmul into the Q psum.
    No per-element bias vector ops remain on the device.
  - Q/K stay bf16 for the S = K^T Q matmuls (contraction is only 64
    deep -- fp8 DoubleRow does not apply -- and bf16 runs at the same
    1 cycle/row while keeping S accurate).
  - The softmax exp of the [1024, 1024] S matrix per head (8.4M
    elements -- the single largest engine workload) is split between
    the Activation engine (exact exp -> fp8 out) and the Vector engine
    (a one-instruction Schraudolph bit-trick: int8(S*A + B) aliased as
    fp8e4m3 approximates exp(S/8) to ~3% RMS, well inside the 2e-2
    tolerance).
  - Softmax denominators come free as ones-columns of V^T in the AV
    matmul. GPSIMD cannot touch PSUM and no engine has a divide, so:
    mid-kernel heads ACT-copy the AV psum to SBUF, take a one-op
    bitwise reciprocal of the denominator row on Pool, bounce it
    through DRAM into a 64-partition broadcast, and Pool multiplies;
    the two tail heads instead get a PE-replicated denominator (ones
    lhsT), an exact DVE reciprocal and a Pool multiply, keeping the
    end-of-kernel chain short.
  - The residual + folded biases ride into the output-projection psum
    via a bf16 identity matmul, so the final drain is a plain copy
    split across ACT/DVE, written back as bf16 (host casts to f32).
"""

import sys

if "/opt/trn_rl_repo" not in sys.path:
    sys.path.insert(0, "/opt/trn_rl_repo")

import numpy as np
import ml_dtypes

import concourse.bass as bass
import concourse.mybir as mybir
from concourse.tile import TileContext

DIM = 512
NH = 8
HD = 64
N = 1024
P = 128
CT = DIM // P   # 4 c-tiles of 128 channels
JT = N // P     # 8 j-tiles of 128 positions
F32 = mybir.dt.float32
BF16 = mybir.dt.bfloat16
F8 = mybir.dt.float8e4
I8 = mybir.dt.int8
I32 = mybir.dt.int32
AOP = mybir.AluOpType
EXP = mybir.ActivationFunctionType.Exp
DRM = mybir.MatmulPerfMode.DoubleRow

# Schraudolph exp: fp8e4m3 byte ~= int8(S * SCH_A + SCH_B)  approximates
# exp(S * 0.125). B tuned numerically for truncating float->int casts.
SCH_A = 8.0 / np.log(2.0) * 0.125
SCH_B = 56.08

# GPSIMD (Pool) cannot touch PSUM (BIR verifier rule), so every psum
# drain runs on ACT or DVE; Pool only gets SBUF->SBUF work (denominator
# broadcast + normalize divide on the ACT-copied AV output).

# Exp engine split: within each jt the two head-tiles go to DIFFERENT
# engines (h2=0 -> ACT, h2=1 -> DVE) so both engines chew in parallel on
# the psS bufs; extra h2=0 tiles shift to DVE for load balance.
EXP_DVE = {(p, 1, jt) for p in range(4) for jt in range(8)}

# engine for each Q/K psum->bf16 copy, by (tensor, ot)
QK_COPY_ENG = {("q", 0): "V", ("k", 0): "A", ("q", 1): "V", ("k", 1): "A",
               ("q", 2): "V", ("k", 2): "A", ("q", 3): "V", ("k", 3): "A"}
# engine for the final psum->bf16 output copies, per (ot, nh)
RESID_ENG = {(0, 0): "A", (0, 1): "V", (1, 0): "A", (1, 1): "V",
             (2, 0): "A", (2, 1): "V", (3, 0): "A", (3, 1): "V"}

# head -> (g, s, half) slot in the output-projection rhs. The host
# permutes Wo's input-channel order to match (see _prep_maps). Chosen so
# the tail heads h6/h7 land in lower halves (direct engine write, no
# partition-remap DMA on the critical tail); the remapped (half=1) heads
# h1/h3/h4/h5 all complete mid-kernel.
HEAD_SLOT = {0: (0, 0, 0), 1: (0, 0, 1), 2: (0, 1, 0), 3: (0, 1, 1),
             4: (1, 0, 1), 5: (1, 1, 1), 6: (1, 1, 0), 7: (1, 0, 0)}


class FixedTileContext(TileContext):
    """Works around a walrus/bass snapshot mismatch: this walrus build
    accepts only one sync-wait command per instruction, but Tile's wait
    assigner happily attaches several. After scheduling, excess waits on
    any instruction are peeled off onto same-engine NOPs inserted right
    before it (same blocking semantics: the engine executes in order)."""

    MAX_WAITS = 1
    MAX_WAITS_DATA = 1
    _wsplit_ctr = 0

    def _split_sync_waits(self):
        seq_only = mybir.SEQUENCER_ONLY_OPCODES
        for fn in self.nc.m.functions:
            for blk in fn.blocks:
                insts = list(blk.instructions)
                out = []
                for inst in insts:
                    si = inst.sync_info
                    limit = (
                        self.MAX_WAITS
                        if inst.opcode in seq_only
                        else self.MAX_WAITS_DATA
                    )
                    if si is not None and len(si.on_wait) > limit:
                        waits = list(si.on_wait)
                        movers = waits[:-limit]
                        keep = waits[-limit:]
                        del si.on_wait[:]
                        for w in keep:
                            si.on_wait.append(w)
                        for w in movers:
                            FixedTileContext._wsplit_ctr += 1
                            nop = mybir.InstNoOp(
                                name=f"wsplit-{FixedTileContext._wsplit_ctr}",
                                ins=[],
                                outs=[],
                            )
                            nop.engine = inst.engine
                            nop.sync_info = mybir.SyncInfo(on_wait=[w], on_update=[])
                            out.append(nop)
                    out.append(inst)
                if len(out) != len(insts):
                    del blk.instructions[:]
                    for i in out:
                        blk.add_instruction(i)

    split_on_exit = True

    def __exit__(self, *exc):
        ret = super().__exit__(*exc)
        if exc[0] is None and self.split_on_exit:
            self._split_sync_waits()
        return ret


def build_nc(split_waits=True):
    nc = bass.Bass()

    x8d = nc.dram_tensor("x8", [P, CT, N], F8, kind="ExternalInput")
    xrd = nc.dram_tensor("xr16", [P, CT, N], BF16, kind="ExternalInput")
    i16d = nc.dram_tensor("i16", [P, P], BF16, kind="ExternalInput")
    wq8d = nc.dram_tensor("wq8", [P, CT, DIM], F8, kind="ExternalInput")
    wk8d = nc.dram_tensor("wk8", [P, CT, DIM], F8, kind="ExternalInput")
    wv8d = nc.dram_tensor("wv8", [P, CT, DIM], F8, kind="ExternalInput")
    wo8d = nc.dram_tensor("wo8", [P, CT, DIM], F8, kind="ExternalInput")
    bq8d = nc.dram_tensor("bq8", [1, 2, DIM], F8, kind="ExternalInput")
    on8d = nc.dram_tensor("on8", [1, 2, N], F8, kind="ExternalInput")
    outd = nc.dram_tensor("out", [DIM, N], BF16, kind="ExternalOutput")
    outr = outd.rearrange("(t p) n -> t p n", p=P)

    FixedTileContext.split_on_exit = split_waits
    with FixedTileContext(nc) as tc:
        with (
            tc.tile_pool(name="persist", bufs=1) as persist,
            tc.tile_pool(name="small", bufs=5) as small,
            tc.tile_pool(name="dram", bufs=1, space="DRAM") as dram,
            tc.tile_pool(name="obf", bufs=8) as obf,
            tc.tile_pool(name="psS", bufs=2, space="PSUM") as psS,
        ):
            # persistent SBUF tensors
            x8 = persist.tile([P, CT, N], F8, tag="x8", name="x8")
            xr16 = persist.tile([P, CT, N], BF16, tag="xr16", name="xr16")
            i16 = persist.tile([P, P], BF16, tag="i16", name="i16")
            wq8 = persist.tile([P, CT, DIM], F8, tag="wq8", name="wq8")
            wk8 = persist.tile([P, CT, DIM], F8, tag="wk8", name="wk8")
            wv8 = persist.tile([P, CT, DIM], F8, tag="wv8", name="wv8")
            wo8 = persist.tile([P, CT, DIM], F8, tag="wo8", name="wo8")
            bq8 = persist.tile([1, 2, DIM], F8, tag="bq8", name="bq8")
            on8 = persist.tile([1, 2, N], F8, tag="on8", name="on8")
            qt = [persist.tile([P, N], BF16, tag=f"q_{t}", name=f"q_{t}")
                  for t in range(CT)]
            kt = [persist.tile([P, N], BF16, tag=f"k_{t}", name=f"k_{t}")
                  for t in range(CT)]
            vt = [persist.tile([P, 2, NH, HD + 2], F8, tag=f"v_{a}",
                               name=f"v_{a}") for a in range(CT)]
            p8 = {}
            for a in range(CT):
                for h in range(NH):
                    p8[(a, h)] = persist.tile(
                        [P, 2, N], F8, tag=f"p8_{a}_{h}", name=f"p8_{a}_{h}")
            o8 = [persist.tile([P, 2, N], F8, tag=f"o8_{g}", name=f"o8_{g}")
                  for g in range(2)]
            ones8 = persist.tile([P, 2, HD], F8, tag="ones8", name="ones8")
            nc.vector.memset(ones8, 1.0)
            # magic constant for the one-op bitwise reciprocal approx
            # (r ~= bitcast(0x7EF127EA - bits(x)), ~4% max rel err -- the
            # softmax denominator tolerates it easily)
            cmagic = persist.tile([HD + 1, 512], I32, tag="cmagic",
                                  name="cmagic")
            nc.vector.memset(cmagic, 0x7EF127EA)

            def copy_psum(eng, out_ap, in_ap):
                if eng == "A":
                    nc.scalar.copy(out_ap, in_ap)
                else:
                    nc.vector.tensor_copy(out_ap, in_ap)

            def exp_op(key, dst_ap, src_ap, split=False):
                if split:
                    # both engines drain one tile: ACT the low half (exact
                    # exp), DVE the high half (Schraudolph) -- used for the
                    # tail pair so the last tiles clear in half the time
                    nc.scalar.activation(dst_ap[:, 0:512], src_ap[:, 0:512],
                                         EXP, scale=0.125)
                    nc.vector.tensor_scalar(
                        dst_ap[:, 512:1024].bitcast(I8), src_ap[:, 512:1024],
                        SCH_A, SCH_B, AOP.mult, AOP.add)
                elif key in EXP_DVE:
                    nc.vector.tensor_scalar(dst_ap.bitcast(I8), src_ap,
                                            SCH_A, SCH_B, AOP.mult, AOP.add)
                else:
                    nc.scalar.activation(dst_ap, src_ap, EXP, scale=0.125)

            # S tiles cycle through 3 slots (2 in psS + 1 in psS2 once the
            # projection pool has closed) so both exp engines always have a
            # tile in flight and a third is being refilled by the PE.
            s_slot = [0]

            def s_tile(p, h2, jt, pools, split=False):
                h = 2 * p + h2
                pool, ptag = pools[s_slot[0] % len(pools)]
                s_slot[0] += 1
                t = pool.tile([P, N], F32, tag=ptag, name=f"s_{p}_{h2}_{jt}")
                for ih in range(2):
                    nc.tensor.matmul(
                        t[:, ih * 512:(ih + 1) * 512],
                        lhsT=kt[p][64 * h2:64 * h2 + 64, jt * P:(jt + 1) * P],
                        rhs=qt[p][64 * h2:64 * h2 + 64, ih * 512:(ih + 1) * 512],
                        start=True, stop=True)
                exp_op((p, h2, jt), p8[(jt // 2, h)][:, jt % 2, :], t, split)

            with tc.tile_pool(name="pp", bufs=2, space="PSUM") as pp:
                # ---- input DMAs ---- (ACT queue kept clear for copies/exps)
                nc.sync.dma_start(out=wq8, in_=wq8d[:, :, :])
                nc.sync.dma_start(out=x8[:, 0:2, :], in_=x8d[:, 0:2, :])
                nc.gpsimd.dma_start(out=x8[:, 2:4, :], in_=x8d[:, 2:4, :])
                nc.sync.dma_start(out=wk8, in_=wk8d[:, :, :])
                nc.scalar.dma_start(out=bq8, in_=bq8d[:, :, :])
                nc.scalar.dma_start(out=on8, in_=on8d[:, :, :])
                nc.sync.dma_start(out=wv8, in_=wv8d[:, :, :])

                # warm the exp table on ACT before the first real exp
                # (costless in the cost model, but keeps real hw from paying
                # the table load inside the first pipelined exp)
                pass

                def proj_qk(w8, ot, with_bias, eng, name):
                    ps = pp.tile([P, N], F32, tag="pp", name=f"pp_{name}{ot}")
                    for nh in range(2):
                        sl = slice(nh * 512, (nh + 1) * 512)
                        for g in range(2):
                            nc.tensor.matmul(
                                ps[:, sl],
                                lhsT=w8[:, 2 * g:2 * g + 2, ot * P:(ot + 1) * P],
                                rhs=x8[:, 2 * g:2 * g + 2, sl],
                                start=(g == 0),
                                stop=(g == 1),
                                perf_mode=DRM)
                        if with_bias:
                            nc.tensor.matmul(
                                ps[:, sl],
                                lhsT=bq8[:, :, ot * P:(ot + 1) * P],
                                rhs=on8[:, :, sl],
                                start=False, stop=True, perf_mode=DRM,
                                skip_group_check=True)
                    dst = qt[ot] if name == "q" else kt[ot]
                    copy_psum(eng, dst[:, :], ps[:, :])

                def proj_v(a):
                    ps = pp.tile([P, N], F32, tag="pp", name=f"pp_v{a}")
                    for jloc in range(2):
                        jt = 2 * a + jloc
                        sl = slice(jloc * 512, (jloc + 1) * 512)
                        for g in range(2):
                            nc.tensor.matmul(
                                ps[:, sl],
                                lhsT=x8[:, 2 * g:2 * g + 2, jt * P:(jt + 1) * P],
                                rhs=wv8[:, 2 * g:2 * g + 2, :],
                                start=(g == 0), stop=(g == 1), perf_mode=DRM)
                    psv = ps.rearrange("p (s h d) -> p s h d", s=2, h=NH)
                    if a % 2 == 0:
                        nc.scalar.copy(vt[a][:, :, :, 0:HD], psv)
                    else:
                        nc.vector.tensor_copy(vt[a][:, :, :, 0:HD], psv)
                    nc.vector.memset(vt[a][:, :, :, HD:HD + 2], 1.0)

                proj_qk(wq8, 0, True, QK_COPY_ENG[("q", 0)], "q")
                proj_qk(wk8, 0, False, QK_COPY_ENG[("k", 0)], "k")
                for jt in range(2):
                    s_tile(0, 0, jt, [(psS, "psS")])
                    s_tile(0, 1, jt, [(psS, "psS")])
                for a in range(2):
                    proj_v(a)
                proj_qk(wq8, 1, True, QK_COPY_ENG[("q", 1)], "q")
                proj_qk(wk8, 1, False, QK_COPY_ENG[("k", 1)], "k")
                for jt in range(2, JT):
                    s_tile(0, 0, jt, [(psS, "psS")])
                    s_tile(0, 1, jt, [(psS, "psS")])
                proj_qk(wq8, 2, True, QK_COPY_ENG[("q", 2)], "q")
                proj_qk(wk8, 2, False, QK_COPY_ENG[("k", 2)], "k")
                for a in range(2, CT):
                    proj_v(a)
                proj_qk(wq8, 3, True, QK_COPY_ENG[("q", 3)], "q")
                proj_qk(wk8, 3, False, QK_COPY_ENG[("k", 3)], "k")

                # late loads for the output stage
                nc.gpsimd.dma_start(out=wo8, in_=wo8d[:, :, :])
                nc.sync.dma_start(out=xr16, in_=xrd[:, :, :])
                nc.sync.dma_start(out=i16, in_=i16d[:, :])

            def av_head(h, psO, rdram):
                # Mid-kernel heads: ACT drains the AV psum (head output +
                # denominator row) to SBUF; the denominator row bounces
                # through DRAM to become a 64-partition broadcast (DMA APs
                # allow the 0-stride partition dim engines reject) and the
                # normalize divide runs SBUF-only on Pool.
                g, s, half = HEAD_SLOT[h]
                odd = half == 1
                osc = None
                if odd:
                    osc = small.tile([HD, N], F8, tag="osc", name=f"osc_{h}")
                for ih in range(2):
                    sl = slice(ih * 512, (ih + 1) * 512)
                    po = psO.tile([HD + 2, 512], F32, tag="psO",
                                  name=f"po_{h}_{ih}")
                    for a in range(CT):
                        nc.tensor.matmul(
                            po, lhsT=vt[a][:, :, h, :],
                            rhs=p8[(a, h)][:, :, sl],
                            start=(a == 0), stop=(a == CT - 1),
                            perf_mode=DRM, skip_group_check=True)
                    oraw = small.tile([HD + 1, 512], F32, tag="oraw",
                                      name=f"or_{h}_{ih}")
                    nc.scalar.copy(oraw[:, :], po[0:HD + 1, :])
                    # bit-trick reciprocal of the denominator row on Pool,
                    # bounced through DRAM into a 64-partition broadcast
                    nc.gpsimd.tensor_tensor(
                        oraw[HD:HD + 1, :].bitcast(I32),
                        cmagic[HD:HD + 1, :],
                        oraw[HD:HD + 1, :].bitcast(I32), AOP.subtract)
                    r = 2 * h + ih
                    nc.sync.dma_start(out=rdram[r:r + 1, :],
                                      in_=oraw[HD:HD + 1, :])
                    rb = small.tile([HD, 512], F32, tag="rb",
                                    name=f"rb_{h}_{ih}")
                    rsrc = rdram[r:r + 1, :]
                    nc.sync.dma_start(
                        out=rb[:, :],
                        in_=bass.AP(tensor=rsrc.tensor, offset=rsrc.offset,
                                    ap=[[0, HD]] + list(rsrc.ap[1:])))
                    dst_ap = osc[:, sl] if odd else o8[g][0:HD, s, sl]
                    nc.gpsimd.tensor_tensor(dst_ap, oraw[0:HD, :], rb[:, :],
                                            AOP.mult)
                if odd:
                    nc.sync.dma_start(out=o8[g][HD:P, s, :], in_=osc[:, :])

            def av_tail(h, psO, den_pool, den_tag):
                # Tail heads (h6/h7): the denominator broadcast comes from
                # the PE itself -- an all-ones lhsT DoubleRow accumulation
                # replicates the column sums across 64 psum partitions --
                # so the critical chain is just ACT-copy + DVE divide, with
                # no DRAM bounce latency.
                g, s, _ = HEAD_SLOT[h]
                td = den_pool.tile([P, N], F32, tag=den_tag, name=f"td_{h}")
                for ih in range(2):
                    sl = slice(ih * 512, (ih + 1) * 512)
                    po = psO.tile([HD + 2, 512], F32, tag="psO",
                                  name=f"po_{h}_{ih}")
                    for a in range(CT):
                        nc.tensor.matmul(
                            po, lhsT=vt[a][:, :, h, :],
                            rhs=p8[(a, h)][:, :, sl],
                            start=(a == 0), stop=(a == CT - 1),
                            perf_mode=DRM, skip_group_check=True)
                        nc.tensor.matmul(
                            td[0:HD, sl], lhsT=ones8[:, :, :],
                            rhs=p8[(a, h)][:, :, sl],
                            start=(a == 0), stop=(a == CT - 1),
                            perf_mode=DRM, skip_group_check=True)
                    den = small.tile([HD, 512], F32, tag="rb",
                                     name=f"den_{h}_{ih}")
                    nc.scalar.copy(den[:, :], td[0:HD, sl])
                    nc.vector.tensor_tensor(o8[g][0:HD, s, sl], po[0:HD, :],
                                            den[:, :], AOP.divide)

            def op_mm(ot, g, ps):
                # one g-layer of the output projection for both n-halves;
                # g-major emission lets the g0 layers run (and warm the PE
                # p-state) while the last heads' normalize is still going.
                for nh in range(2):
                    sl = slice(nh * 512, (nh + 1) * 512)
                    nc.tensor.matmul(
                        ps[:, sl],
                        lhsT=wo8[:, 2 * g:2 * g + 2, ot * P:(ot + 1) * P],
                        rhs=o8[g][:, :, sl],
                        start=(g == 0), stop=(g == 1),
                        perf_mode=DRM, skip_group_check=True)

            def op_resid_mm(ot, ps):
                # residual + bias ride the psum via a bf16 identity matmul,
                # so the drain is a plain copy any engine can do
                for nh in range(2):
                    sl = slice(nh * 512, (nh + 1) * 512)
                    nc.tensor.matmul(
                        ps[:, sl], lhsT=i16[:, :], rhs=xr16[:, ot, sl],
                        start=False, stop=True, skip_group_check=True)

            def op_tail(ot, ps):
                # psum (already holding proj + residual + bias) -> bf16 ->
                # out; ACT and DVE each drain one half into a shared buffer
                # so the writeback is a single DMA per ot (HWDGE holds are
                # 625ns each and serialize -- fewer, bigger DMAs win)
                ob = obf.tile([P, N], BF16, tag="ob", name=f"ob_{ot}")
                nc.scalar.copy(ob[:, 0:512], ps[:, 0:512])
                nc.vector.tensor_copy(ob[:, 512:1024], ps[:, 512:1024])
                nc.sync.dma_start(out=outr[ot], in_=ob)

            with (
                tc.tile_pool(name="psS2", bufs=1, space="PSUM") as psS2,
                tc.tile_pool(name="psO", bufs=2, space="PSUM") as psO,
            ):
                pools3 = [(psS, "psS"), (psS, "psS"), (psS2, "psS2")]
                for jt in range(JT):
                    s_tile(1, 0, jt, pools3)
                    s_tile(1, 1, jt, pools3)
                rdram = dram.tile([12, 512], F32, tag="rdram", name="rdram")
                av_head(0, psO, rdram)
                av_head(1, psO, rdram)
                for jt in range(JT):
                    s_tile(2, 0, jt, pools3)
                    s_tile(2, 1, jt, pools3)
                av_head(2, psO, rdram)
                av_head(3, psO, rdram)
                for jt in range(JT):
                    s_tile(3, 1, jt, pools3, split=True)
                    s_tile(3, 0, jt, pools3, split=True)
                    if jt == 1:
                        av_head(4, psO, rdram)
                    elif jt == 4:
                        av_head(5, psO, rdram)
                ps0 = psS.tile([P, N], F32, tag="psS", name="op_0")
                ps1 = psS.tile([P, N], F32, tag="psS", name="op_1")
                op_mm(0, 0, ps0)
                op_mm(1, 0, ps1)
                op_resid_mm(0, ps0)
                op_resid_mm(1, ps1)
                av_tail(7, psO, psS2, "psS2")
                av_tail(6, psO, psS2, "psS2")
                op_mm(0, 1, ps0)
                op_mm(1, 1, ps1)
                op_tail(0, ps0)
                op_tail(1, ps1)
            with tc.tile_pool(name="po3", bufs=2, space="PSUM") as po3:
                ps2 = po3.tile([P, N], F32, tag="po3", name="op_2")
                ps3 = po3.tile([P, N], F32, tag="po3", name="op_3")
                op_mm(2, 0, ps2)
                op_mm(3, 0, ps3)
                op_mm(2, 1, ps2)
                op_mm(3, 1, ps3)
                op_resid_mm(2, ps2)
                op_resid_mm(3, ps3)
                op_tail(2, ps2)
                op_tail(3, ps3)
    return nc


_BF = ml_dtypes.bfloat16
_E4 = ml_dtypes.float8_e4m3


def _prep_maps(x, Wq, bq, Wk, bk, Wv, bv, Wo, bo):
    # plain numpy up front: inputs may arrive as jax device arrays and
    # transforming those would trigger on-device jax execution
    x, Wq, bq, Wk, bk, Wv, bv, Wo, bo = (
        np.asarray(a, dtype=np.float32)
        for a in (x, Wq, bq, Wk, bk, Wv, bv, Wo, bo)
    )
    B, C, H, W = x.shape
    xf = np.ascontiguousarray(x.reshape(B, C, H * W))

    def wtile(Wm):
        # [128, CT, DIM] with [p, t, o] = W[o, 128t + p], fp8
        return np.ascontiguousarray(
            Wm.T.reshape(CT, P, DIM).transpose(1, 0, 2)).astype(_E4)

    # Wo's input channels are permuted to match the device's o8 head-slot
    # layout (HEAD_SLOT): channel 128*(2g+s) + 64*half + d <- head H's d.
    cperm = np.zeros(DIM, np.int64)
    for h, (g, s, half) in HEAD_SLOT.items():
        base = P * (2 * g + s) + HD * half
        cperm[base:base + HD] = HD * h + np.arange(HD)
    Wo_p = Wo[:, cperm]

    bo_p = bo + Wo @ bv  # bv folded through the output projection (exact)
    bq8 = np.zeros((1, 2, DIM), np.float32)
    bq8[0, 0, :] = 64.0 * bq
    on8 = np.zeros((1, 2, N), np.float32)
    on8[0, 0, :] = 1.0 / 64.0
    shared = {
        "wq8": wtile(Wq), "wk8": wtile(Wk), "wv8": wtile(Wv),
        "wo8": wtile(Wo_p), "bq8": bq8.astype(_E4), "on8": on8.astype(_E4),
        "i16": np.eye(P, dtype=np.float32).astype(_BF),
    }
    in_maps = []
    for b in range(B):
        m = dict(shared)
        xb = xf[b]                                  # [C, N]
        m["x8"] = np.ascontiguousarray(
            xb.reshape(CT, P, N).transpose(1, 0, 2)).astype(_E4)
        m["xr16"] = np.ascontiguousarray(
            (xb + bo_p[:, None]).reshape(CT, P, N).transpose(1, 0, 2)
        ).astype(_BF)
        in_maps.append(m)
    return in_maps


def kernel(x, Wq, bq, Wk, bk, Wv, bv, Wo, bo, _trace=False):
    from concourse.bass_utils import run_bass_kernel_spmd

    x = np.asarray(x)
    B, C, H, W = x.shape
    in_maps = _prep_maps(x, Wq, bq, Wk, bk, Wv, bv, Wo, bo)
    nc = build_nc()
    res = run_bass_kernel_spmd(nc, in_maps, core_ids=list(range(B)), trace=_trace)
    out = np.stack([res.results[b]["out"] for b in range(B)])
    out = out.reshape(B, C, H, W).astype(np.float32)
    if _trace:
        kernel.last_results = res
    return out
